# revision 2
# baseline (speedup 1.0000x reference)
"""BiLSTM-CRF loss kernel for 8 Trainium2 NeuronCores.

Sharding: direction x batch split. Cores 0-3 run the forward LSTM on batch
slices of 16 sequences; cores 4-7 run the backward LSTM (same program, with
time-reversed inputs). Per core: input projection (big matmul), 512-step
recurrence (PE matmuls + ACT/DVE gate math), output projection to partial
emission features.

Orchestration (the part that matters for wall time): everything except the
LSTM recurrence runs as cached jax jits on the same NeuronCores —
  jit_prep : embedding gather + transpose -> xT per core (device-side)
  jit_bass : the Bass program via the bass_exec custom call (cached jit,
             weights resident on device across calls)
  jit_crf  : feats assembly + CRF forward algorithm + loss (device-side)
Warm calls upload only sentence/tags/mask (~0.5 MB) and download one scalar.
"""

import numpy as np
import ml_dtypes

import jax
import jax.numpy as jnp
from jax.scipy.special import logsumexp
from jax.sharding import Mesh, PartitionSpec as P, NamedSharding
from jax.experimental.shard_map import shard_map

import concourse.bass as bass
import concourse.mybir as mybir
import concourse.tile as tile
from concourse import bacc, bass2jax

BF16 = ml_dtypes.bfloat16

B, L, V, E, HD, T = 64, 512, 32000, 512, 1024, 10
H = HD // 2          # 512 per-direction hidden
G4 = 4 * H           # 2048 gate rows
BL = 16              # sequences per core (64 batch / 4 slices; dirs split 0-3/4-7)
NC = L * BL          # 8192 (t-major columns: col = t*BL + b)
KC = H // 128        # 4 contraction chunks
MC = G4 // 128       # 16 gate-row chunks
NB = NC // 512       # 16 column blocks for the input projection

F32 = mybir.dt.float32
BF16_T = mybir.dt.bfloat16
AF = mybir.ActivationFunctionType

_cache = {}


def _build_program(steps=L):
    nc = bacc.Bacc("TRN2", target_bir_lowering=False, debug=False, num_devices=8)

    xT = nc.dram_tensor("xT", [E, NC], BF16_T, kind="ExternalInput").ap()
    w_ihT = nc.dram_tensor("w_ihT", [E, G4], BF16_T, kind="ExternalInput").ap()
    w_hhT = nc.dram_tensor("w_hhT", [H, G4], BF16_T, kind="ExternalInput").ap()
    bias_pm = nc.dram_tensor("bias_pm", [128, MC], F32, kind="ExternalInput").ap()
    w_outT = nc.dram_tensor("w_outT", [H, T], BF16_T, kind="ExternalInput").ap()
    feats = nc.dram_tensor("feats", [T, NC], F32, kind="ExternalOutput").ap()
    pre = nc.dram_tensor("pre", [MC, 128, NC], F32).ap()  # scratch in DRAM

    with tile.TileContext(nc) as tc:
        with (
            tc.tile_pool(name="singles", bufs=1) as singles,
            tc.tile_pool(name="xin", bufs=1) as xin,
            tc.tile_pool(name="psA", bufs=4, space="PSUM") as psA,
            tc.tile_pool(name="evA", bufs=4) as evA,
            tc.tile_pool(name="prestream", bufs=4) as prestream,
            tc.tile_pool(name="psB", bufs=2, space="PSUM") as psB,
            tc.tile_pool(name="gtmp", bufs=2) as gtmp,
            tc.tile_pool(name="atmp", bufs=2) as atmp,
            tc.tile_pool(name="stmp", bufs=3) as stmp,
            tc.tile_pool(name="psF", bufs=2, space="PSUM") as psFp,
            tc.tile_pool(name="evF", bufs=2) as evFp,
        ):
            # ---- resident weights ----
            wih_sb = [singles.tile([128, G4], BF16_T, tag=f"wih{k}", name=f"wih{k}") for k in range(KC)]
            whh_sb = [singles.tile([128, G4], BF16_T, tag=f"whh{k}", name=f"whh{k}") for k in range(KC)]
            for k in range(KC):
                nc.sync.dma_start(out=wih_sb[k], in_=w_ihT[128 * k:128 * (k + 1), :])
                nc.sync.dma_start(out=whh_sb[k], in_=w_hhT[128 * k:128 * (k + 1), :])
            bias_sb = singles.tile([128, MC], F32, tag="bias")
            nc.sync.dma_start(out=bias_sb, in_=bias_pm)
            wout_sb = [singles.tile([128, T], BF16_T, tag=f"wo{k}", name=f"wo{k}") for k in range(KC)]
            for k in range(KC):
                nc.sync.dma_start(out=wout_sb[k], in_=w_outT[128 * k:128 * (k + 1), :])

            # ---- phase A: pre-gates = W_ih @ x (+bias), streamed to DRAM ----
            xk_sb = [xin.tile([128, NC], BF16_T, tag=f"x{k}", name=f"x{k}") for k in range(KC)]
            for k in range(KC):
                nc.sync.dma_start(out=xk_sb[k], in_=xT[128 * k:128 * (k + 1), :])
            for m in range(MC):
                for nb in range(NB):
                    ps = psA.tile([128, 512], F32)
                    for k in range(KC):
                        nc.tensor.matmul(
                            ps,
                            wih_sb[k][:, 128 * m:128 * (m + 1)],
                            xk_sb[k][:, 512 * nb:512 * (nb + 1)],
                            start=(k == 0), stop=(k == KC - 1),
                        )
                    ev = evA.tile([128, 512], F32)
                    nc.scalar.activation(ev, ps, AF.Identity,
                                         bias=bias_sb[:, m:m + 1])
                    nc.sync.dma_start(out=pre[m, :, 512 * nb:512 * (nb + 1)], in_=ev)

            # ---- phase B: recurrence ----
            # h history: [128, KC, (steps+1)*BL] bf16; col block s holds h_{s-1}
            hh = singles.tile([128, KC, (steps + 1) * BL], BF16_T, tag="hh")
            nc.vector.memset(hh[:, :, 0:BL], 0.0)
            c_sb = singles.tile([128, KC * BL], F32, tag="c")
            nc.vector.memset(c_sb, 0.0)

            for t in range(steps):
                pt = prestream.tile([128, MC * BL], F32)
                for mg in range(4):  # 4 DMAs x 4 m-chunks each
                    src = pre.rearrange("m p c -> p m c")[
                        :, 4 * mg:4 * (mg + 1), BL * t:BL * (t + 1)]
                    nc.sync.dma_start(
                        out=pt.rearrange("p (m b) -> p m b", m=MC)[
                            :, 4 * mg:4 * (mg + 1), :],
                        in_=src)
                ps = psB.tile([128, MC * BL], F32)
                hprev = hh[:, :, BL * t:BL * (t + 1)]  # [128, KC, BL]
                for m in range(MC):
                    for k in range(KC):
                        nc.tensor.matmul(
                            ps[:, BL * m:BL * (m + 1)],
                            whh_sb[k][:, 128 * m:128 * (m + 1)],
                            hprev[:, k, :],
                            start=(k == 0), stop=(k == KC - 1),
                        )
                g_sb = gtmp.tile([128, MC * BL], F32)
                # i,f block ready after m=7; g,o after m=15
                nc.vector.tensor_add(g_sb[:, 0:128], ps[:, 0:128], pt[:, 0:128])
                nc.vector.tensor_add(g_sb[:, 128:256], ps[:, 128:256], pt[:, 128:256])
                a_sb = atmp.tile([128, MC * BL], F32)
                nc.scalar.activation(a_sb[:, 0:128], g_sb[:, 0:128], AF.Sigmoid)
                nc.scalar.activation(a_sb[:, 128:192], g_sb[:, 128:192], AF.Tanh)
                nc.scalar.activation(a_sb[:, 192:256], g_sb[:, 192:256], AF.Sigmoid)
                t1 = stmp.tile([128, 64], F32, tag="t1")
                nc.vector.tensor_mul(t1, a_sb[:, 0:64], a_sb[:, 128:192])
                nc.vector.tensor_mul(c_sb, a_sb[:, 64:128], c_sb)
                nc.vector.tensor_add(c_sb, c_sb, t1)
                tcn = stmp.tile([128, 64], F32, tag="tc")
                nc.scalar.activation(tcn, c_sb, AF.Tanh)
                hout = hh[:, :, BL * (t + 1):BL * (t + 2)]
                nc.vector.tensor_mul(
                    hout,
                    a_sb[:, 192:256].rearrange("p (j b) -> p j b", j=KC),
                    tcn.rearrange("p (j b) -> p j b", j=KC),
                )

            # ---- phase C: partial feats = w_out_half.T @ h ----
            ncols_h = steps * BL
            cblk = min(512, ncols_h)
            for nb in range(ncols_h // cblk):
                psF = psFp.tile([T, cblk], F32)
                for k in range(KC):
                    nc.tensor.matmul(
                        psF,
                        wout_sb[k],
                        hh[:, k, BL + cblk * nb:BL + cblk * (nb + 1)],
                        start=(k == 0), stop=(k == KC - 1),
                    )
                evF = evFp.tile([T, cblk], F32)
                nc.vector.tensor_copy(evF, psF)
                nc.sync.dma_start(out=feats[:, cblk * nb:cblk * (nb + 1)], in_=evF)

    nc.compile()
    return nc


def _get_state():
    if "state" in _cache:
        return _cache["state"]

    bass2jax.install_neuronx_cc_hook()
    nc = _build_program()

    devices = jax.devices()[:8]
    mesh = Mesh(np.asarray(devices), ("core",))
    repl = NamedSharding(mesh, P())
    shard0 = NamedSharding(mesh, P("core"))

    # ---- bass_exec callable (mirrors run_bass_via_pjrt, built once) ----
    partition_name = nc.partition_id_tensor.name if nc.partition_id_tensor else None
    in_names, out_names, out_avals, zero_shapes = [], [], [], []
    for alloc in nc.m.functions[0].allocations:
        if not isinstance(alloc, mybir.MemoryLocationSet):
            continue
        name = alloc.memorylocations[0].name
        if alloc.kind == "ExternalInput":
            if name != partition_name:
                in_names.append(name)
        elif alloc.kind == "ExternalOutput":
            shape = tuple(alloc.tensor_shape)
            dtype = mybir.dt.np(alloc.dtype)
            out_names.append(name)
            out_avals.append(jax.core.ShapedArray(shape, dtype))
            zero_shapes.append((shape, dtype))
    n_params = len(in_names)
    n_outs = len(out_avals)
    all_in_names = list(in_names) + list(out_names)
    if partition_name is not None:
        all_in_names.append(partition_name)

    def _body(*args):
        operands = list(args)
        if partition_name is not None:
            operands.append(bass2jax.partition_id_tensor())
        outs = bass2jax._bass_exec_p.bind(
            *operands,
            out_avals=tuple(out_avals),
            in_names=tuple(all_in_names),
            out_names=tuple(out_names),
            lowering_input_output_aliases=(),
            sim_require_finite=True,
            sim_require_nnan=True,
            nc=nc,
        )
        return tuple(outs)

    donate = tuple(range(n_params, n_params + n_outs))
    jit_bass = jax.jit(
        shard_map(_body, mesh=mesh,
                  in_specs=(P("core"),) * (n_params + n_outs),
                  out_specs=(P("core"),) * n_outs, check_rep=False),
        donate_argnums=donate, keep_unused=True,
    )

    def _zeros():
        return tuple(jnp.zeros((8 * s[0], *s[1:]), d) for s, d in zero_shapes)

    jit_zeros = jax.jit(_zeros, out_shardings=(shard0,) * n_outs)

    # ---- device-side prep: embedding gather + transpose per core ----
    def _prep(emb_l, ints_l):
        cid = jax.lax.axis_index("core")
        c = jnp.remainder(cid, 4)
        sent = jax.lax.dynamic_slice(ints_l[0], (c * BL, 0), (BL, L))  # [BL, L]
        sent = jnp.where(cid >= 4, sent[:, ::-1], sent)
        x = emb_l[sent]                                  # [BL, L, E] bf16
        return x.transpose(2, 1, 0).reshape(E, L * BL)   # col = t*BL + b

    jit_prep = jax.jit(
        shard_map(_prep, mesh=mesh, in_specs=(P(), P()),
                  out_specs=P("core"), check_rep=False))

    # ---- device-side CRF ----
    def _crf(feats_l, ints_l, params_l):
        fg = jax.lax.all_gather(feats_l, "core")         # [8, T, NC]
        fg = fg.reshape(8, T, L, BL)
        f = fg[:4] + fg[4:, :, ::-1, :]                  # [4, T, L, BL]
        feats = f.transpose(2, 0, 3, 1).reshape(L, B, T)
        trans = params_l[:T * T].reshape(T, T)
        start = params_l[T * T:T * T + T]
        end = params_l[T * T + T:T * T + 2 * T]
        b_out = params_l[T * T + 2 * T:T * T + 3 * T]
        feats = feats + b_out
        tags = ints_l[1]
        maskT = ints_l[2].T.astype(jnp.float32)          # [L, B]
        tagsT = tags.T                                   # [L, B]
        em = jnp.take_along_axis(feats, tagsT[:, :, None], axis=2)[..., 0]
        score = start[tagsT[0]] + em[0]
        tr = trans[tagsT[:-1], tagsT[1:]]
        score = score + ((tr + em[1:]) * maskT[1:]).sum(axis=0)
        last = ints_l[2].sum(axis=1) - 1
        last_tags = jnp.take_along_axis(tags, last[:, None], axis=1)[:, 0]
        score = score + end[last_tags]
        alpha0 = start[None, :] + feats[0]
        def step(alpha, inp):
            emis, m = inp
            nxt = logsumexp(alpha[:, :, None] + trans[None, :, :]
                            + emis[:, None, :], axis=1)
            return jnp.where(m[:, None] > 0, nxt, alpha), None
        alpha, _ = jax.lax.scan(step, alpha0, (feats[1:], maskT[1:]))
        denom = logsumexp(alpha + end[None, :], axis=1)
        return -((score - denom).sum() / maskT.sum())

    jit_crf = jax.jit(
        shard_map(_crf, mesh=mesh, in_specs=(P("core"), P(), P()),
                  out_specs=P(), check_rep=False))

    state = dict(nc=nc, mesh=mesh, repl=repl, shard0=shard0,
                 in_names=in_names, jit_bass=jit_bass, jit_zeros=jit_zeros,
                 jit_prep=jit_prep, jit_crf=jit_crf)
    _cache["state"] = state
    return state


def _dev_cached(key, srcs, make):
    ent = _cache.get(key)
    if ent is not None and len(ent[0]) == len(srcs) \
            and all(a is b for a, b in zip(ent[0], srcs)):
        return ent[1]
    val = make()
    _cache[key] = (tuple(srcs), val)
    return val


def kernel(sentence, tags, mask, emb, w_ih_f, w_hh_f, b_f,
           w_ih_b, w_hh_b, b_b, w_out, b_out,
           start_trans, end_trans, transitions):
    st = _get_state()
    repl, shard0 = st["repl"], st["shard0"]

    # --- per-call small uploads: sentence/tags/mask packed as one array ---
    ints = np.empty((3, B, L), np.int32)
    ints[0] = np.asarray(sentence)
    ints[1] = np.asarray(tags)
    ints[2] = np.asarray(mask)
    ints_d = jax.device_put(ints, repl)

    params = np.concatenate([
        np.asarray(transitions, np.float32).ravel(),
        np.asarray(start_trans, np.float32).ravel(),
        np.asarray(end_trans, np.float32).ravel(),
        np.asarray(b_out, np.float32).ravel(),
    ])
    params_d = jax.device_put(params, repl)

    # --- device-resident (identity-cached) big constants ---
    def _mk_emb():
        return jax.device_put(np.asarray(emb, np.float32).astype(BF16), repl)
    emb_d = _dev_cached("emb", (emb,), _mk_emb)

    def _mk_weights():
        def cat(per_core):
            return np.concatenate(per_core, axis=0)
        wih = {}
        for nm, wf, wb in (("w_ihT", w_ih_f, w_ih_b), ("w_hhT", w_hh_f, w_hh_b)):
            f = np.ascontiguousarray(np.asarray(wf, np.float32).T).astype(BF16)
            b = np.ascontiguousarray(np.asarray(wb, np.float32).T).astype(BF16)
            wih[nm] = jax.device_put(cat([f] * 4 + [b] * 4), shard0)
        bf = np.ascontiguousarray(np.asarray(b_f, np.float32).reshape(MC, 128).T)
        bb = np.ascontiguousarray(np.asarray(b_b, np.float32).reshape(MC, 128).T)
        wih["bias_pm"] = jax.device_put(cat([bf] * 4 + [bb] * 4), shard0)
        wo = np.asarray(w_out, np.float32)
        wof = np.ascontiguousarray(wo[:, :H].T).astype(BF16)
        wob = np.ascontiguousarray(wo[:, H:].T).astype(BF16)
        wih["w_outT"] = jax.device_put(cat([wof] * 4 + [wob] * 4), shard0)
        return wih
    weights = _dev_cached("weights", (w_ih_f, w_hh_f, b_f, w_ih_b, w_hh_b,
                                      b_b, w_out), _mk_weights)

    # --- device pipeline: prep -> bass -> crf (async dispatch) ---
    xT_g = st["jit_prep"](emb_d, ints_d)
    arg_map = dict(weights)
    arg_map["xT"] = xT_g
    args = [arg_map[nm] for nm in st["in_names"]]
    zeros = st["jit_zeros"]()
    outs = st["jit_bass"](*args, *zeros)
    loss = st["jit_crf"](outs[0], ints_d, params_d)
    return np.float32(np.asarray(loss))


# revision 11
# speedup vs baseline: 1.9792x; 1.9792x over previous
"""BiLSTM-CRF loss kernel for 8 Trainium2 NeuronCores.

Sharding: direction x batch split. Cores 0-3 run the forward LSTM on batch
slices of 16 sequences; cores 4-7 run the backward LSTM (same program, with
time-reversed inputs). Per core: input projection (big matmul), 512-step
recurrence (PE matmuls + ACT/DVE gate math), output projection to partial
emission features.

Orchestration (the part that matters for wall time): everything except the
LSTM recurrence runs as cached jax jits on the same NeuronCores —
  jit_prep : embedding gather + transpose -> xT per core (device-side)
  jit_bass : the Bass program via the bass_exec custom call (cached jit,
             weights resident on device across calls)
  jit_crf  : feats assembly + CRF forward algorithm + loss (device-side)
Warm calls upload only sentence/tags/mask (~0.5 MB) and download one scalar.
"""

import numpy as np
import ml_dtypes

import jax
import jax.numpy as jnp
from jax.scipy.special import logsumexp
from jax.sharding import Mesh, PartitionSpec as P, NamedSharding
from jax.experimental.shard_map import shard_map

import concourse.bass as bass
import concourse.mybir as mybir
import concourse.tile as tile
from concourse import bacc, bass2jax

BF16 = ml_dtypes.bfloat16

B, L, V, E, HD, T = 64, 512, 32000, 512, 1024, 10
H = HD // 2          # 512 per-direction hidden
G4 = 4 * H           # 2048 gate rows
BL = 16              # sequences per core (64 batch / 4 slices; dirs split 0-3/4-7)
NC = L * BL          # 8192 (t-major columns: col = t*BL + b)
KC = H // 128        # 4 contraction chunks
MC = G4 // 128       # 16 gate-row chunks
NB = NC // 512       # 16 column blocks for the input projection

F32 = mybir.dt.float32
BF16_T = mybir.dt.bfloat16
AF = mybir.ActivationFunctionType

_cache = {}


def _build_program(steps=L):
    nc = bacc.Bacc("TRN2", target_bir_lowering=False, debug=False, num_devices=8)

    xT = nc.dram_tensor("xT", [E, NC], BF16_T, kind="ExternalInput").ap()
    w_ihT = nc.dram_tensor("w_ihT", [E, G4], BF16_T, kind="ExternalInput").ap()
    w_hhT = nc.dram_tensor("w_hhT", [H, G4], BF16_T, kind="ExternalInput").ap()
    bias_pm = nc.dram_tensor("bias_pm", [128, MC], F32, kind="ExternalInput").ap()
    w_outT = nc.dram_tensor("w_outT", [H, T], BF16_T, kind="ExternalInput").ap()
    feats = nc.dram_tensor("feats", [T, NC], F32, kind="ExternalOutput").ap()
    pre = nc.dram_tensor("pre", [MC, 128, NC], F32).ap()  # scratch in DRAM

    with tile.TileContext(nc) as tc:
        with (
            tc.tile_pool(name="singles", bufs=1) as singles,
            tc.tile_pool(name="xin", bufs=1) as xin,
            tc.tile_pool(name="psA", bufs=4, space="PSUM") as psA,
            tc.tile_pool(name="evA", bufs=4) as evA,
            tc.tile_pool(name="prestream", bufs=4) as prestream,
            tc.tile_pool(name="psB", bufs=2, space="PSUM") as psB,
            tc.tile_pool(name="gtmp", bufs=2) as gtmp,
            tc.tile_pool(name="atmp", bufs=2) as atmp,
            tc.tile_pool(name="stmp", bufs=3) as stmp,
            tc.tile_pool(name="psF", bufs=2, space="PSUM") as psFp,
            tc.tile_pool(name="evF", bufs=2) as evFp,
        ):
            # ---- resident weights ----
            wih_sb = [singles.tile([128, G4], BF16_T, tag=f"wih{k}", name=f"wih{k}") for k in range(KC)]
            whh_sb = [singles.tile([128, G4], BF16_T, tag=f"whh{k}", name=f"whh{k}") for k in range(KC)]
            for k in range(KC):
                nc.sync.dma_start(out=wih_sb[k], in_=w_ihT[128 * k:128 * (k + 1), :])
                nc.sync.dma_start(out=whh_sb[k], in_=w_hhT[128 * k:128 * (k + 1), :])
            bias_sb = singles.tile([128, MC], F32, tag="bias")
            nc.sync.dma_start(out=bias_sb, in_=bias_pm)
            wout_sb = [singles.tile([128, T], BF16_T, tag=f"wo{k}", name=f"wo{k}") for k in range(KC)]
            for k in range(KC):
                nc.sync.dma_start(out=wout_sb[k], in_=w_outT[128 * k:128 * (k + 1), :])

            # ---- phase A: pre-gates = W_ih @ x (+bias), streamed to DRAM ----
            xk_sb = [xin.tile([128, NC], BF16_T, tag=f"x{k}", name=f"x{k}") for k in range(KC)]
            for k in range(KC):
                nc.sync.dma_start(out=xk_sb[k], in_=xT[128 * k:128 * (k + 1), :])
            for m in range(MC):
                for nb in range(NB):
                    ps = psA.tile([128, 512], F32)
                    for k in range(KC):
                        nc.tensor.matmul(
                            ps,
                            wih_sb[k][:, 128 * m:128 * (m + 1)],
                            xk_sb[k][:, 512 * nb:512 * (nb + 1)],
                            start=(k == 0), stop=(k == KC - 1),
                        )
                    ev = evA.tile([128, 512], F32)
                    nc.scalar.activation(ev, ps, AF.Identity,
                                         bias=bias_sb[:, m:m + 1])
                    nc.sync.dma_start(out=pre[m, :, 512 * nb:512 * (nb + 1)], in_=ev)

            # ---- phase B: recurrence ----
            # h history: [128, KC, (steps+1)*BL] bf16; col block s holds h_{s-1}
            hh = singles.tile([128, KC, (steps + 1) * BL], BF16_T, tag="hh")
            nc.vector.memset(hh[:, :, 0:BL], 0.0)
            c_sb = singles.tile([128, KC * BL], F32, tag="c")
            nc.vector.memset(c_sb, 0.0)

            for t in range(steps):
                pt = prestream.tile([128, MC * BL], F32)
                for mg in range(4):  # 4 DMAs x 4 m-chunks each
                    src = pre.rearrange("m p c -> p m c")[
                        :, 4 * mg:4 * (mg + 1), BL * t:BL * (t + 1)]
                    nc.sync.dma_start(
                        out=pt.rearrange("p (m b) -> p m b", m=MC)[
                            :, 4 * mg:4 * (mg + 1), :],
                        in_=src)
                ps = psB.tile([128, MC * BL], F32)
                hprev = hh[:, :, BL * t:BL * (t + 1)]  # [128, KC, BL]
                for m in range(MC):
                    for k in range(KC):
                        nc.tensor.matmul(
                            ps[:, BL * m:BL * (m + 1)],
                            whh_sb[k][:, 128 * m:128 * (m + 1)],
                            hprev[:, k, :],
                            start=(k == 0), stop=(k == KC - 1),
                        )
                g_sb = gtmp.tile([128, MC * BL], F32)
                # i,f block ready after m=7; g,o after m=15
                nc.vector.tensor_add(g_sb[:, 0:128], ps[:, 0:128], pt[:, 0:128])
                nc.vector.tensor_add(g_sb[:, 128:256], ps[:, 128:256], pt[:, 128:256])
                a_sb = atmp.tile([128, MC * BL], F32)
                nc.scalar.activation(a_sb[:, 0:128], g_sb[:, 0:128], AF.Sigmoid)
                nc.scalar.activation(a_sb[:, 128:192], g_sb[:, 128:192], AF.Tanh)
                nc.scalar.activation(a_sb[:, 192:256], g_sb[:, 192:256], AF.Sigmoid)
                t1 = stmp.tile([128, 64], F32, tag="t1")
                nc.vector.tensor_mul(t1, a_sb[:, 0:64], a_sb[:, 128:192])
                nc.vector.tensor_mul(c_sb, a_sb[:, 64:128], c_sb)
                nc.vector.tensor_add(c_sb, c_sb, t1)
                tcn = stmp.tile([128, 64], F32, tag="tc")
                nc.scalar.activation(tcn, c_sb, AF.Tanh)
                hout = hh[:, :, BL * (t + 1):BL * (t + 2)]
                nc.vector.tensor_mul(
                    hout,
                    a_sb[:, 192:256].rearrange("p (j b) -> p j b", j=KC),
                    tcn.rearrange("p (j b) -> p j b", j=KC),
                )

            # ---- phase C: partial feats = w_out_half.T @ h ----
            ncols_h = steps * BL
            cblk = min(512, ncols_h)
            for nb in range(ncols_h // cblk):
                psF = psFp.tile([T, cblk], F32)
                for k in range(KC):
                    nc.tensor.matmul(
                        psF,
                        wout_sb[k],
                        hh[:, k, BL + cblk * nb:BL + cblk * (nb + 1)],
                        start=(k == 0), stop=(k == KC - 1),
                    )
                evF = evFp.tile([T, cblk], F32)
                nc.vector.tensor_copy(evF, psF)
                nc.sync.dma_start(out=feats[:, cblk * nb:cblk * (nb + 1)], in_=evF)

    nc.compile()
    return nc


def _get_state():
    if "state" in _cache:
        return _cache["state"]

    bass2jax.install_neuronx_cc_hook()
    nc = _build_program()

    devices = jax.devices()[:8]
    mesh = Mesh(np.asarray(devices), ("core",))
    repl = NamedSharding(mesh, P())
    shard0 = NamedSharding(mesh, P("core"))

    # ---- bass_exec callable (mirrors run_bass_via_pjrt, built once) ----
    partition_name = nc.partition_id_tensor.name if nc.partition_id_tensor else None
    in_names, out_names, out_avals, zero_shapes = [], [], [], []
    for alloc in nc.m.functions[0].allocations:
        if not isinstance(alloc, mybir.MemoryLocationSet):
            continue
        name = alloc.memorylocations[0].name
        if alloc.kind == "ExternalInput":
            if name != partition_name:
                in_names.append(name)
        elif alloc.kind == "ExternalOutput":
            shape = tuple(alloc.tensor_shape)
            dtype = mybir.dt.np(alloc.dtype)
            out_names.append(name)
            out_avals.append(jax.core.ShapedArray(shape, dtype))
            zero_shapes.append((shape, dtype))
    n_params = len(in_names)
    n_outs = len(out_avals)
    all_in_names = list(in_names) + list(out_names)
    if partition_name is not None:
        all_in_names.append(partition_name)

    def _body(*args):
        operands = list(args)
        if partition_name is not None:
            operands.append(bass2jax.partition_id_tensor())
        outs = bass2jax._bass_exec_p.bind(
            *operands,
            out_avals=tuple(out_avals),
            in_names=tuple(all_in_names),
            out_names=tuple(out_names),
            lowering_input_output_aliases=(),
            sim_require_finite=True,
            sim_require_nnan=True,
            nc=nc,
        )
        return tuple(outs)

    donate = tuple(range(n_params, n_params + n_outs))
    jit_bass = jax.jit(
        shard_map(_body, mesh=mesh,
                  in_specs=(P("core"),) * (n_params + n_outs),
                  out_specs=(P("core"),) * n_outs, check_rep=False),
        donate_argnums=donate, keep_unused=True,
    )

    NI = 3 * B * L  # int32 words of sentence/tags/mask in the flat upload

    # ---- device-side prep: embedding gather + transpose per core ----
    # Also returns the zero-filled output buffers donated to jit_bass, so
    # no separate zeros dispatch (and no host upload) is needed.
    def _prep(emb_l, flat_l):
        cid = jax.lax.axis_index("core")
        c = jnp.remainder(cid, 4)
        sent_all = flat_l[:B * L].reshape(B, L)
        sent = jax.lax.dynamic_slice(sent_all, (c * BL, 0), (BL, L))
        sent = jnp.where(cid >= 4, sent[:, ::-1], sent)
        x = emb_l[sent]                                  # [BL, L, E] bf16
        xT = x.transpose(2, 1, 0).reshape(E, L * BL)     # col = t*BL + b
        zs = tuple(jnp.zeros(s, d) for s, d in zero_shapes)
        return (xT,) + zs

    jit_prep = jax.jit(
        shard_map(_prep, mesh=mesh, in_specs=(P(), P()),
                  out_specs=(P("core"),) * (1 + n_outs), check_rep=False))

    # ---- device-side CRF ----
    def _crf(feats_l, flat_l):
        fg = jax.lax.all_gather(feats_l, "core")         # [8, T, NC]
        fg = fg.reshape(8, T, L, BL)
        f = fg[:4] + fg[4:, :, ::-1, :]                  # [4, T, L, BL]
        feats = f.transpose(2, 0, 3, 1).reshape(L, B, T)
        params_l = jax.lax.bitcast_convert_type(flat_l[NI:], jnp.float32)
        trans = params_l[:T * T].reshape(T, T)
        start = params_l[T * T:T * T + T]
        end = params_l[T * T + T:T * T + 2 * T]
        b_out = params_l[T * T + 2 * T:T * T + 3 * T]
        feats = feats + b_out
        ints_l = flat_l[:NI].reshape(3, B, L)
        tags = ints_l[1]
        maskT = ints_l[2].T.astype(jnp.float32)          # [L, B]
        tagsT = tags.T                                   # [L, B]
        em = jnp.take_along_axis(feats, tagsT[:, :, None], axis=2)[..., 0]
        score = start[tagsT[0]] + em[0]
        tr = trans[tagsT[:-1], tagsT[1:]]
        score = score + ((tr + em[1:]) * maskT[1:]).sum(axis=0)
        last = ints_l[2].sum(axis=1) - 1
        last_tags = jnp.take_along_axis(tags, last[:, None], axis=1)[:, 0]
        score = score + end[last_tags]

        alpha0 = start[None, :] + feats[0]
        def step(alpha, inp):
            emis, m = inp
            nxt = logsumexp(alpha[:, :, None] + trans[None, :, :]
                            + emis[:, None, :], axis=1)
            return jnp.where(m[:, None] > 0, nxt, alpha), None
        alpha, _ = jax.lax.scan(step, alpha0, (feats[1:], maskT[1:]),
                                unroll=8)
        denom = logsumexp(alpha + end[None, :], axis=1)
        return -((score - denom).sum() / maskT.sum())

    jit_crf = jax.jit(
        shard_map(_crf, mesh=mesh, in_specs=(P("core"), P()),
                  out_specs=P(), check_rep=False))

    state = dict(nc=nc, mesh=mesh, repl=repl, shard0=shard0, NI=NI,
                 in_names=in_names, jit_bass=jit_bass,
                 jit_prep=jit_prep, jit_crf=jit_crf)
    _cache["state"] = state
    return state


def _same(a, b):
    if a is b:
        return True
    a = np.asarray(a)
    b = np.asarray(b)
    return a.shape == b.shape and a.dtype == b.dtype and np.array_equal(a, b)


def _dev_cached(key, srcs, make):
    ent = _cache.get(key)
    if ent is not None and len(ent[0]) == len(srcs) \
            and all(_same(a, b) for a, b in zip(ent[0], srcs)):
        return ent[1]
    val = make()
    _cache[key] = (tuple(srcs), val)
    return val


def kernel(sentence, tags, mask, emb, w_ih_f, w_hh_f, b_f,
           w_ih_b, w_hh_b, b_b, w_out, b_out,
           start_trans, end_trans, transitions):
    st = _get_state()
    repl, shard0, NI = st["repl"], st["shard0"], st["NI"]

    # --- one small per-call upload: sentence/tags/mask + bitcast f32 params ---
    flat = np.empty(NI + T * T + 3 * T, np.int32)
    iv = flat[:NI].reshape(3, B, L)
    iv[0] = np.asarray(sentence)
    iv[1] = np.asarray(tags)
    iv[2] = np.asarray(mask)
    flat[NI:] = np.concatenate([
        np.asarray(transitions, np.float32).ravel(),
        np.asarray(start_trans, np.float32).ravel(),
        np.asarray(end_trans, np.float32).ravel(),
        np.asarray(b_out, np.float32).ravel(),
    ]).view(np.int32)
    ent = _cache.get("flat")
    if ent is not None and np.array_equal(ent[0], flat):
        flat_d = ent[1]          # identical content: device copy is still valid
    else:
        flat_d = jax.device_put(flat, repl)
        _cache["flat"] = (flat, flat_d)

    # --- device-resident (identity-cached) big constants ---
    def _mk_emb():
        return jax.device_put(np.asarray(emb, np.float32).astype(BF16), repl)
    emb_d = _dev_cached("emb", (emb,), _mk_emb)

    def _mk_weights():
        def cat(per_core):
            return np.concatenate(per_core, axis=0)
        wih = {}
        for nm, wf, wb in (("w_ihT", w_ih_f, w_ih_b), ("w_hhT", w_hh_f, w_hh_b)):
            f = np.ascontiguousarray(np.asarray(wf, np.float32).T).astype(BF16)
            b = np.ascontiguousarray(np.asarray(wb, np.float32).T).astype(BF16)
            wih[nm] = jax.device_put(cat([f] * 4 + [b] * 4), shard0)
        bf = np.ascontiguousarray(np.asarray(b_f, np.float32).reshape(MC, 128).T)
        bb = np.ascontiguousarray(np.asarray(b_b, np.float32).reshape(MC, 128).T)
        wih["bias_pm"] = jax.device_put(cat([bf] * 4 + [bb] * 4), shard0)
        wo = np.asarray(w_out, np.float32)
        wof = np.ascontiguousarray(wo[:, :H].T).astype(BF16)
        wob = np.ascontiguousarray(wo[:, H:].T).astype(BF16)
        wih["w_outT"] = jax.device_put(cat([wof] * 4 + [wob] * 4), shard0)
        return wih
    weights = _dev_cached("weights", (w_ih_f, w_hh_f, b_f, w_ih_b, w_hh_b,
                                      b_b, w_out), _mk_weights)

    # --- device pipeline: prep -> bass -> crf (async dispatch) ---
    prep_out = st["jit_prep"](emb_d, flat_d)
    xT_g, zeros = prep_out[0], prep_out[1:]
    arg_map = dict(weights)
    arg_map["xT"] = xT_g
    args = [arg_map[nm] for nm in st["in_names"]]
    outs = st["jit_bass"](*args, *zeros)
    loss = st["jit_crf"](outs[0], flat_d)
    return np.float32(np.asarray(loss))
